# revision 23
# baseline (speedup 1.0000x reference)
"""ChordMixerBlock Trainium2 kernel.

Math (per batch b):
    h   = gelu(data @ w1 + b1)            # exact gelu
    y   = h @ w2 + b2
    out[l, :] = rotate_chord(y)[l, :] + data[l, :]
where rotate_chord rolls track t (channels [16t, 16t+16)) forward by
s_t = 2^(t-1) positions along L (track 0: no shift; track 15: 2^14 == L
-> no shift).

Sharding: 8 cores = (batch b, L-half j); each core computes y for its own
8192-token chunk in transposed layout [256 d, 8192 l] so the contraction
dim D lands on SBUF partitions (host pre-transposes inputs and transposes
the output back).

Roll handling: acc[c, p] = y[c, p] + b2[c] + dataS[c, p], where dataS is
the residual pre-rolled by +s_t per track on the HOST (pure layout prep).
acc[c, p] is then exactly out[global (c0 + p - s_t) mod L, c].  The
kernel dumps acc columns RAW as large contiguous blocks; the host undoes
the per-track rotation during unshard with np.roll over the two chunks
concatenated along L (pure indexing; wraps across the chunk boundary are
handled exactly by the roll).

Schedule: l-tiles are processed in groups of G=2 with weight-stationary
matmul ordering and a software pipeline fc1(g+1) ahead of fc2(g).
Strictly alternating fc1/fc2 on the PE is essential: gelus recycle the
ph banks during fc2's matmuls; two consecutive fc1 groups would outrun
the ACT engine and stall the PE.  All bulk input goes on the SWDGE
(Pool) ring, which is the fastest DMA path; stores alternate between
the SP HWDGE ring and the SWDGE ring.
"""

import sys

sys.path.insert(0, "/opt/trn_rl_repo")

import numpy as np
import ml_dtypes

import concourse.bass as bass
import concourse.bacc as bacc
import concourse.tile as tile
import concourse.mybir as mybir
from concourse import bass_utils

B, L, D, H = 4, 16384, 256, 512
N_CORES = 8
LC = L // 2                      # per-core chunk length
NT, TS = 16, 16                  # tracks, track size
SHIFTS = [0] + [2 ** i for i in range(NT - 1)]
SEFF = [s % L for s in SHIFTS]   # track 15 -> 0
TILE = 512                       # l-tile width for matmuls (one PSUM bank)
NTILES = LC // TILE
G = 2                            # l-tiles per weight-stationary group
NG = NTILES // G
BLK = 2048                       # DMA block width
NBLK = LC // BLK

F32 = mybir.dt.float32
BF16 = mybir.dt.bfloat16


def _build():
    nc = bacc.Bacc(
        "TRN2", target_bir_lowering=False, debug=False,
        num_devices=N_CORES,
    )

    dataM_h = nc.dram_tensor("dataM", [D, LC], BF16, kind="ExternalInput")
    dataS_h = nc.dram_tensor("dataS", [D, LC], BF16, kind="ExternalInput")
    w1_h = nc.dram_tensor("w1b", [D, H], BF16, kind="ExternalInput")
    w2_h = nc.dram_tensor("w2b", [H, D], BF16, kind="ExternalInput")
    b1_h = nc.dram_tensor("b1m", [128, H // 128], F32, kind="ExternalInput")
    b2_h = nc.dram_tensor("b2m", [128, D // 128], F32, kind="ExternalInput")
    outT_h = nc.dram_tensor("outT", [D, LC], BF16, kind="ExternalOutput")

    with tile.TileContext(nc) as tc:
        with (
            tc.tile_pool(name="const", bufs=1) as cpool,
            tc.tile_pool(name="big", bufs=1) as big,
            tc.tile_pool(name="hbf", bufs=20) as hbfp,
            tc.tile_pool(name="ph", bufs=4, space="PSUM") as php,
            tc.tile_pool(name="py", bufs=4, space="PSUM") as pyp,
        ):
            # --- weights / biases on the SP ring (parallel to the SWDGE
            # input stream; putting w1 at the SWDGE head instead delays
            # the first dataM pieces, which are the binding constraint) ---
            w1sb = []
            for dt in range(2):
                w = cpool.tile([128, H], BF16, tag=f"w1_{dt}", name=f"w1sb{dt}")
                nc.sync.dma_start(w[:], w1_h.ap()[dt * 128:(dt + 1) * 128, :])
                w1sb.append(w)
            b1sb = cpool.tile([128, H // 128], F32, tag="b1")
            nc.sync.dma_start(b1sb[:], b1_h.ap())
            w2sb = []
            for ht in range(4):
                w = cpool.tile([128, D], BF16, tag=f"w2_{ht}", name=f"w2sb{ht}")
                nc.sync.dma_start(w[:], w2_h.ap()[ht * 128:(ht + 1) * 128, :])
                w2sb.append(w)
            b2sb = cpool.tile([128, D // 128], F32, tag="b2")
            nc.sync.dma_start(b2sb[:], b2_h.ap())

            # --- persistent chunk buffers ---
            dm = [big.tile([128, LC], BF16, tag=f"dm{k}", name=f"dm{k}")
                  for k in range(2)]
            ds = [big.tile([128, LC], BF16, tag=f"ds{k}", name=f"ds{k}")
                  for k in range(2)]
            acc = [big.tile([128, LC], BF16, tag=f"acc{k}", name=f"acc{k}")
                   for k in range(2)]

            # inputs on the SWDGE (Pool) ring so the HWDGE rings stay free;
            # dataM block first (matmuls need it), then the matching dataS
            # block (the fc2 epilogue needs it a little later)
            # the input rings deliver nothing before ~9us (queue startup)
            # and ~0.25 MiB/us after, so the ramp is ring-limited: feed the
            # first dataM block in 512-col pieces paced to the PE's
            # consumption order (g0 runs u-outer), and push dataS blk0 to
            # the otherwise-idle ACT ring so the SWDGE ring can deliver
            # dataM blk1 before fc1(g1) needs it
            for lo, hi in ((0, TILE), (TILE, 2 * TILE)):
                for k in range(2):
                    rows = slice(k * 128, (k + 1) * 128)
                    nc.gpsimd.dma_start(
                        dm[k][:, lo:hi], dataM_h.ap()[rows, lo:hi])
            for k in range(2):
                rows = slice(k * 128, (k + 1) * 128)
                nc.gpsimd.dma_start(
                    dm[k][:, 2 * TILE:BLK], dataM_h.ap()[rows, 2 * TILE:BLK])
            for k in range(2):
                rows = slice(k * 128, (k + 1) * 128)
                nc.scalar.dma_start(ds[k][:, 0:BLK], dataS_h.ap()[rows, 0:BLK])
            for blk in range(1, NBLK):
                sl = slice(blk * BLK, (blk + 1) * BLK)
                for k in range(2):
                    rows = slice(k * 128, (k + 1) * 128)
                    nc.gpsimd.dma_start(dm[k][:, sl], dataM_h.ap()[rows, sl])
                for k in range(2):
                    rows = slice(k * 128, (k + 1) * 128)
                    nc.gpsimd.dma_start(ds[k][:, sl], dataS_h.ap()[rows, sl])

            # alternate block stores over the SP (HWDGE) and Pool (SWDGE)
            # rings; the ACT ring stays reserved for the gelu stream
            _oc = [0]

            def out_eng():
                _oc[0] += 1
                return nc.sync if _oc[0] % 2 else nc.gpsimd

            def emit_fc1(g):
                """fc1 + gelu for tiles [g*G, (g+1)*G); returns hbf[u][ht]."""
                hbf = [[None] * 4 for _ in range(G)]
                for ht in range(4):
                    hs = slice(ht * 128, (ht + 1) * 128)
                    phs = [php.tile([128, TILE], F32, tag="ph",
                                    name=f"ph_{g}_{ht}_{u}")
                           for u in range(G)]
                    # group 0 runs u-outer so the matmul order matches the
                    # arrival order of the 512-col prologue DMA pieces
                    order = (
                        [(dt, u) for u in range(G) for dt in range(2)]
                        if g == 0 else
                        [(dt, u) for dt in range(2) for u in range(G)]
                    )
                    for dt, u in order:
                        i = g * G + u
                        csl = slice(i * TILE, (i + 1) * TILE)
                        nc.tensor.matmul(
                            phs[u][:], w1sb[dt][:, hs], dm[dt][:, csl],
                            start=(dt == 0), stop=(dt == 1),
                        )
                    for u in range(G):
                        hb = hbfp.tile([128, TILE], BF16, tag="hbf",
                                       name=f"hbf_{g}_{ht}_{u}")
                        nc.scalar.activation(
                            hb[:], phs[u][:],
                            mybir.ActivationFunctionType.Gelu,
                            bias=b1sb[:, ht:ht + 1],
                        )
                        hbf[u][ht] = hb
                return hbf

            def emit_fc2(g, hbf):
                for k in range(2):
                    pys = [pyp.tile([128, TILE], F32, tag="py",
                                    name=f"py_{g}_{k}_{u}")
                           for u in range(G)]
                    for ht in range(4):
                        for u in range(G):
                            nc.tensor.matmul(
                                pys[u][:],
                                w2sb[ht][:, k * 128:(k + 1) * 128],
                                hbf[u][ht][:],
                                start=(ht == 0), stop=(ht == 3),
                            )
                    for u in range(G):
                        i = g * G + u
                        csl = slice(i * TILE, (i + 1) * TILE)
                        # acc = (y + b2) + rolled residual
                        nc.vector.scalar_tensor_tensor(
                            acc[k][:, csl], pys[u][:], b2sb[:, k:k + 1],
                            ds[k][:, csl],
                            mybir.AluOpType.add, mybir.AluOpType.add,
                        )

                done = (g + 1) * G * TILE
                if g == NG - 1:
                    # per-tile stores for the last group: the final serial
                    # piece after the last STT is only 128 KB
                    for k in range(2):
                        for u in range(G):
                            i = g * G + u
                            sl = slice(i * TILE, (i + 1) * TILE)
                            out_eng().dma_start(
                                outT_h.ap()[k * 128:(k + 1) * 128, sl],
                                acc[k][:, sl],
                            )
                elif g == NG - 2:
                    sl = slice(g * G * TILE, done)
                    for k in range(2):
                        out_eng().dma_start(
                            outT_h.ap()[k * 128:(k + 1) * 128, sl],
                            acc[k][:, sl],
                        )
                elif done % BLK == 0:
                    sl = slice(done - BLK, done)
                    for k in range(2):
                        out_eng().dma_start(
                            outT_h.ap()[k * 128:(k + 1) * 128, sl],
                            acc[k][:, sl],
                        )

            # --- software-pipelined main loop: fc1(g+1) ahead of fc2(g) ---
            prev = None
            for g in range(NG + 1):
                cur = emit_fc1(g) if g < NG else None
                if prev is not None:
                    emit_fc2(g - 1, prev)
                prev = cur

    nc.compile()
    return nc


_NC = None


def _get_nc():
    global _NC
    if _NC is None:
        _NC = _build()
    return _NC


def make_in_maps(data, w1, b1, w2, b2):
    data = np.asarray(data, dtype=np.float32)
    w1b = np.asarray(w1, dtype=np.float32).astype(ml_dtypes.bfloat16)
    w2b = np.asarray(w2, dtype=np.float32).astype(ml_dtypes.bfloat16)
    b1m = np.ascontiguousarray(
        np.asarray(b1, dtype=np.float32).reshape(H // 128, 128).T
    )
    b2m = np.ascontiguousarray(
        np.asarray(b2, dtype=np.float32).reshape(D // 128, 128).T
    )

    in_maps = []
    for bb in range(B):
        # residual pre-rolled by +s_t per track:
        # rolled[l, c] = data[(l - s_t) mod L, c]
        rolled = np.empty((L, D), dtype=np.float32)
        for t in range(NT):
            cs = slice(t * TS, (t + 1) * TS)
            rolled[:, cs] = np.roll(data[bb, :, cs], SEFF[t], axis=0)
        for j in range(2):
            sl = slice(j * LC, (j + 1) * LC)
            dataM = np.ascontiguousarray(
                data[bb, sl, :].T.astype(ml_dtypes.bfloat16)
            )
            dataS = np.ascontiguousarray(
                rolled[sl, :].T.astype(ml_dtypes.bfloat16)
            )
            in_maps.append({
                "dataM": dataM, "dataS": dataS,
                "w1b": w1b, "w2b": w2b, "b1m": b1m, "b2m": b2m,
            })
    return in_maps


def kernel(data, w1, b1, w2, b2):
    nc = _get_nc()
    in_maps = make_in_maps(data, w1, b1, w2, b2)
    res = bass_utils.run_bass_kernel_spmd(
        nc, in_maps, core_ids=list(range(N_CORES))
    )
    # acc[c, p] = out[(c0 + p - s_t) mod L, c]: concatenate the two
    # chunks of each batch along L and undo the per-track roll on host
    out = np.empty((B, L, D), dtype=np.float32)
    for bb in range(B):
        glob = np.concatenate(
            [res.results[2 * bb]["outT"], res.results[2 * bb + 1]["outT"]],
            axis=1,
        ).astype(np.float32)                     # [D, L]
        for t in range(NT):
            s = SEFF[t]
            cs = slice(t * TS, (t + 1) * TS)
            if s:
                out[bb, :, cs] = np.roll(glob[cs, :], -s, axis=1).T
            else:
                out[bb, :, cs] = glob[cs, :].T
    return out
